# revision 4
# baseline (speedup 1.0000x reference)
"""AttentionCropLayer kernel for Trainium2 (8 NeuronCores, data parallel).

Math: for each sample, reference does soft-mask + crop + align_corners
bilinear resize to 224x224.  Both the mask and the bilinear sampling are
separable, so the whole thing factors into two small dense matmuls per
channel:

    out[c] = R' @ img[c] @ C'^T
    R'[i,h] = ((1-fr_i)[h==r0_i] + fr_i[h==r1_i]) * mrow[h]   (224x448)
    C'[j,w] = ((1-fc_j)[w==c0_j] + fc_j[w==c1_j]) * mcol[w]   (224x448)

R'/C' depend only on locs (tiny), so the host precomputes them and the
device does the heavy lifting: stream 2.4MB/sample of image data, two
matmuls (+ a PE transpose of the intermediate), write 0.6MB/sample.

Device layout per sample (P=112 partition chunks):
  stage1: tmp[i,w]   = sum_h RT[h,i] * img[h,w]      (lhsT=RT chunk, rhs=img)
  transp: tT[w,i]    = tmp.T                          (PE transpose via identity)
  stage2: out'[j,ci] = sum_w CM[w,j] * tT[w,(c,i)]    (lhsT=CM chunk, rhs=tT)
Output is written j-major ([224, 3*224] per sample); host untransposes.
"""

import sys
from contextlib import ExitStack

import numpy as np

if "/opt/trn_rl_repo" not in sys.path:
    sys.path.insert(0, "/opt/trn_rl_repo")

import concourse.bass as bass  # noqa: E402
import concourse.bacc as bacc  # noqa: E402
import concourse.tile as tile  # noqa: E402
from concourse import mybir  # noqa: E402
from concourse import bass_utils as _bass_utils  # noqa: E402
from concourse.bass_utils import run_bass_kernel_spmd  # noqa: E402
from concourse.masks import make_identity  # noqa: E402


def _install_profile_shims():
    """The agent image's antenv lacks axon_hooks; provide it so trace=True
    can capture NTFF profiles, and stub out the S3 artifact upload."""
    import types

    if "antenv.axon_hooks" not in sys.modules:
        mod = types.ModuleType("antenv.axon_hooks")
        holder = {}
        mod.set_axon_ntff_profile_hook = lambda h: holder.__setitem__("h", h)
        mod.get_axon_ntff_profile_hook = lambda: holder.get("h")
        sys.modules["antenv.axon_hooks"] = mod
        try:
            import antenv

            antenv.axon_hooks = mod
        except ImportError:
            pass
        try:
            from trn_agent_boot.trn_boot import _ntff_profile_via_ctypes

            hook = _ntff_profile_via_ctypes("/opt/axon/libaxon_pjrt.so")
            if hook is not None:
                mod.set_axon_ntff_profile_hook(hook)
        except Exception as e:  # pragma: no cover
            print(f"NTFF hook install failed: {e}", file=sys.stderr)
    _bass_utils.upload_artifacts = lambda tmpdir: f"local:{tmpdir}"


_install_profile_shims()

N_CORES = 8
B = 64
BPC = B // N_CORES  # samples per core
C = 3
H = 448  # input height/width
OUT = 224  # output height/width
P = 112  # partition chunk: 448 = 4*112, 224 = 2*112
F32 = mybir.dt.float32
F32R = mybir.dt.float32r
USE_F32R = False

_CACHE = {}


# ---------------------------------------------------------------- host math
def _sigmoid_f32(x):
    x = x.astype(np.float32)
    with np.errstate(over="ignore", under="ignore", invalid="ignore"):
        pos = 1.0 / (1.0 + np.exp(-np.abs(x), dtype=np.float32))
        ex = np.exp(-np.abs(x), dtype=np.float32)
        neg = ex / (1.0 + ex)
    return np.where(x >= 0, pos, neg).astype(np.float32)


def _interp_matrices(locs):
    """locs [B,3] f32 -> RT [B,448,224] f32 (= R'^T), CM [B,448,224] f32 (= C'^T).

    Mirrors reference.py's float32 arithmetic step by step.
    """
    locs = np.asarray(locs, dtype=np.float32)
    nb = locs.shape[0]
    tx, ty, tl = locs[:, 0], locs[:, 1], locs[:, 2]
    third = np.float32(H / 3.0)
    tl = np.maximum(tl, third)
    tx = np.clip(tx, tl, np.float32(H) - tl)
    ty = np.clip(ty, tl, np.float32(H) - tl)
    w_off = np.clip(np.floor(tx - tl), 0, H).astype(np.int32)
    h_off = np.clip(np.floor(ty - tl), 0, H).astype(np.int32)
    w_end = np.clip(np.floor(tx + tl), 0, H).astype(np.int32)
    h_end = np.clip(np.floor(ty + tl), 0, H).astype(np.int32)

    coord = np.arange(H, dtype=np.float32)[None, :]  # [1,448]
    mrow = _sigmoid_f32(10.0 * (coord - w_off[:, None].astype(np.float32))) - _sigmoid_f32(
        10.0 * (coord - w_end[:, None].astype(np.float32))
    )  # [B,448]
    mcol = _sigmoid_f32(10.0 * (coord - h_off[:, None].astype(np.float32))) - _sigmoid_f32(
        10.0 * (coord - h_end[:, None].astype(np.float32))
    )

    t = (np.arange(OUT, dtype=np.float32) / np.float32(OUT - 1))[None, :]  # [1,224]

    def one_hot_lerp(off, end, mask):
        # sample positions r = off + t*(end-1-off), float32 like the reference
        r = off[:, None].astype(np.float32) + t * (end - 1 - off)[:, None].astype(np.float32)
        r0 = np.floor(r).astype(np.int32)
        r1 = np.clip(r0 + 1, 0, H - 1)
        fr = (r - r0).astype(np.float32)
        m = np.zeros((nb, OUT, H), dtype=np.float32)
        bi = np.arange(nb)[:, None]
        oi = np.arange(OUT)[None, :]
        np.add.at(m, (bi, oi, r0), (1.0 - fr).astype(np.float32))
        np.add.at(m, (bi, oi, r1), fr)
        m *= mask[:, None, :]
        return np.ascontiguousarray(m.transpose(0, 2, 1))  # [B,448,224]

    rt = one_hot_lerp(w_off, w_end, mrow)
    cm = one_hot_lerp(h_off, h_end, mcol)
    return rt, cm


# ---------------------------------------------------------------- device code
def _build_nc():
    nc = bacc.Bacc("TRN2", target_bir_lowering=False, debug=False, num_devices=N_CORES)
    img = nc.dram_tensor("images", [BPC, C, H, H], F32, kind="ExternalInput")
    rt = nc.dram_tensor("rt", [BPC, H, OUT], F32, kind="ExternalInput")
    cm = nc.dram_tensor("cm", [BPC, H, OUT], F32, kind="ExternalInput")
    out = nc.dram_tensor("out", [BPC, OUT, C * OUT], F32, kind="ExternalOutput")

    def mm(ap):
        return ap.bitcast(F32R) if USE_F32R else ap

    with tile.TileContext(nc) as tc, ExitStack() as ctx:
        const_pool = ctx.enter_context(tc.tile_pool(name="const", bufs=1))
        ident = const_pool.tile([128, 128], F32)
        make_identity(nc, ident)

        img_pool = ctx.enter_context(tc.tile_pool(name="imgp", bufs=2))
        mat_pool = ctx.enter_context(tc.tile_pool(name="matp", bufs=2))
        tmp_pool = ctx.enter_context(tc.tile_pool(name="tmpp", bufs=4))
        tT_pool = ctx.enter_context(tc.tile_pool(name="ttp", bufs=2))
        out_pool = ctx.enter_context(tc.tile_pool(name="outp", bufs=2))
        ps1 = ctx.enter_context(tc.tile_pool(name="ps1", bufs=3, space="PSUM"))
        psT = ctx.enter_context(tc.tile_pool(name="psT", bufs=3, space="PSUM"))
        ps2 = ctx.enter_context(tc.tile_pool(name="ps2", bufs=2, space="PSUM"))

        for b in range(BPC):
            img_s = img_pool.tile([P, C, 4, H], F32, tag="img")
            nc.sync.dma_start(
                out=img_s, in_=img[b].rearrange("c (hc p) w -> p c hc w", p=P)
            )
            rt_s = mat_pool.tile([P, 4, OUT], F32, tag="rt")
            nc.sync.dma_start(out=rt_s, in_=rt[b].rearrange("(hc p) i -> p hc i", p=P))
            cm_s = mat_pool.tile([P, 4, OUT], F32, tag="cm")
            nc.sync.dma_start(out=cm_s, in_=cm[b].rearrange("(wc p) j -> p wc j", p=P))

            tT_all = tT_pool.tile([P, 4, C * OUT], F32, tag="tT")  # [w, wc, (c,i)]
            for c in range(C):
                tmp_sb = []
                for ic in range(2):
                    t_ps = ps1.tile([P, H], F32, tag="s1")
                    for hc in range(4):
                        nc.tensor.matmul(
                            t_ps,
                            mm(rt_s[:, hc, ic * P : (ic + 1) * P]),
                            mm(img_s[:, c, hc, :]),
                            start=(hc == 0),
                            stop=(hc == 3),
                        )
                    sb = tmp_pool.tile([P, H], F32, tag="tmp")
                    nc.vector.tensor_copy(out=sb, in_=t_ps)
                    tmp_sb.append(sb)
                for wc in range(4):
                    tp = psT.tile([P, OUT], F32, tag="sT")
                    for ic in range(2):
                        nc.tensor.transpose(
                            tp[:, ic * P : (ic + 1) * P],
                            tmp_sb[ic][:, wc * P : (wc + 1) * P],
                            ident[:P, :P],
                        )
                    nc.scalar.copy(out=tT_all[:, wc, c * OUT : (c + 1) * OUT], in_=tp)

            out_sb = out_pool.tile([P, 2, C * OUT], F32, tag="osb")  # [j, jc, (c,i)]
            NB = C * OUT // 2  # 336
            for jc in range(2):
                for nb_i in range(2):
                    o_ps = ps2.tile([P, NB], F32, tag="s2")
                    for wc in range(4):
                        nc.tensor.matmul(
                            o_ps,
                            mm(cm_s[:, wc, jc * P : (jc + 1) * P]),
                            mm(tT_all[:, wc, nb_i * NB : (nb_i + 1) * NB]),
                            start=(wc == 0),
                            stop=(wc == 3),
                        )
                    nc.vector.tensor_copy(
                        out=out_sb[:, jc, nb_i * NB : (nb_i + 1) * NB], in_=o_ps
                    )
                nc.sync.dma_start(
                    out=out[b, jc * P : (jc + 1) * P, :], in_=out_sb[:, jc, :]
                )

    nc.compile()
    return nc


def _get_nc():
    if "nc" not in _CACHE:
        _CACHE["nc"] = _build_nc()
    return _CACHE["nc"]


# ---------------------------------------------------------------- entry point
def kernel(images, locs, _trace=False):
    images = np.ascontiguousarray(np.asarray(images, dtype=np.float32))
    locs = np.asarray(locs, dtype=np.float32)
    rt, cm = _interp_matrices(locs)

    nc = _get_nc()
    in_maps = []
    for i in range(N_CORES):
        s = slice(i * BPC, (i + 1) * BPC)
        in_maps.append(
            {
                "images": images[s],
                "rt": np.ascontiguousarray(rt[s]),
                "cm": np.ascontiguousarray(cm[s]),
            }
        )
    br = run_bass_kernel_spmd(nc, in_maps, core_ids=list(range(N_CORES)), trace=_trace)
    outs = np.concatenate([r["out"] for r in br.results], axis=0)  # [64,224,672]
    # out'[b, j, (c,i)] -> out[b, c, i, j]
    result = np.ascontiguousarray(outs.reshape(B, OUT, C, OUT).transpose(0, 2, 3, 1))
    if _trace:
        return result, br
    return result


if __name__ == "__main__":
    imgs = np.random.randn(B, C, H, H).astype(np.float32)
    lcs = (np.random.rand(B, 3) * H).astype(np.float32)
    out = kernel(imgs, lcs)
    print("out", out.shape, out.dtype)


# revision 9
# speedup vs baseline: 1.8024x; 1.8024x over previous
"""AttentionCropLayer kernel for Trainium2 (8 NeuronCores, data parallel).

Math: for each sample, reference does soft-mask + crop + align_corners
bilinear resize to 224x224.  Both the mask and the bilinear sampling are
separable, so the whole thing factors into two small dense matmuls per
channel:

    out[c] = R' @ img[c] @ C'^T
    R'[i,h] = ((1-fr_i)[h==r0_i] + fr_i[h==r1_i]) * mrow[h]   (224x448)
    C'[j,w] = ((1-fc_j)[w==c0_j] + fc_j[w==c1_j]) * mcol[w]   (224x448)

R'/C' depend only on locs (tiny), so the host precomputes them and the
device does the heavy lifting: stream 2.4MB/sample of image data, two
matmuls (+ a PE transpose of the intermediate), write 0.6MB/sample.

Device layout per sample (P=112 partition chunks):
  stage1: tmp[i,w]   = sum_h RT[h,i] * img[h,w]      (lhsT=RT chunk, rhs=img)
  transp: tT[w,i]    = tmp.T                          (PE transpose via identity)
  stage2: out'[j,ci] = sum_w CM[w,j] * tT[w,(c,i)]    (lhsT=CM chunk, rhs=tT)
Output is written j-major ([224, 3*224] per sample); host untransposes.
"""

import sys
from contextlib import ExitStack

import numpy as np

if "/opt/trn_rl_repo" not in sys.path:
    sys.path.insert(0, "/opt/trn_rl_repo")

import concourse.bass as bass  # noqa: E402
import concourse.bacc as bacc  # noqa: E402
import concourse.tile as tile  # noqa: E402
from concourse import mybir  # noqa: E402
from concourse import bass_utils as _bass_utils  # noqa: E402
from concourse.bass_utils import run_bass_kernel_spmd  # noqa: E402
from concourse.masks import make_identity  # noqa: E402


def _install_profile_shims():
    """The agent image's antenv lacks axon_hooks; provide it so trace=True
    can capture NTFF profiles, and stub out the S3 artifact upload."""
    import types

    if "antenv.axon_hooks" not in sys.modules:
        mod = types.ModuleType("antenv.axon_hooks")
        holder = {}
        mod.set_axon_ntff_profile_hook = lambda h: holder.__setitem__("h", h)
        mod.get_axon_ntff_profile_hook = lambda: holder.get("h")
        sys.modules["antenv.axon_hooks"] = mod
        try:
            import antenv

            antenv.axon_hooks = mod
        except ImportError:
            pass
        try:
            from trn_agent_boot.trn_boot import _ntff_profile_via_ctypes

            hook = _ntff_profile_via_ctypes("/opt/axon/libaxon_pjrt.so")
            if hook is not None:
                mod.set_axon_ntff_profile_hook(hook)
        except Exception as e:  # pragma: no cover
            print(f"NTFF hook install failed: {e}", file=sys.stderr)
    _bass_utils.upload_artifacts = lambda tmpdir: f"local:{tmpdir}"


_install_profile_shims()

N_CORES = 8
B = 64
BPC = B // N_CORES  # samples per core
C = 3
H = 448  # input height/width
OUT = 224  # output height/width
P = 112  # partition chunk: 448 = 4*112, 224 = 2*112
F32 = mybir.dt.float32
F32R = mybir.dt.float32r
USE_F32R = True

_CACHE = {}


def _round_tf32(a):
    """Round float32 array to tf32 (10-bit mantissa), nearest-even."""
    u = a.view(np.uint32)
    u = (u + np.uint32(0xFFF) + ((u >> np.uint32(13)) & np.uint32(1))) & np.uint32(
        ~np.uint32(0x1FFF)
    )
    return u.view(np.float32)


# ---------------------------------------------------------------- host math
def _sigmoid_f32(x):
    x = x.astype(np.float32)
    with np.errstate(over="ignore", under="ignore", invalid="ignore"):
        pos = 1.0 / (1.0 + np.exp(-np.abs(x), dtype=np.float32))
        ex = np.exp(-np.abs(x), dtype=np.float32)
        neg = ex / (1.0 + ex)
    return np.where(x >= 0, pos, neg).astype(np.float32)


def _interp_matrices(locs):
    """locs [B,3] f32 -> RT [B,448,224] f32 (= R'^T), CM [B,448,224] f32 (= C'^T).

    Mirrors reference.py's float32 arithmetic step by step.
    """
    locs = np.asarray(locs, dtype=np.float32)
    nb = locs.shape[0]
    tx, ty, tl = locs[:, 0], locs[:, 1], locs[:, 2]
    third = np.float32(H / 3.0)
    tl = np.maximum(tl, third)
    tx = np.clip(tx, tl, np.float32(H) - tl)
    ty = np.clip(ty, tl, np.float32(H) - tl)
    w_off = np.clip(np.floor(tx - tl), 0, H).astype(np.int32)
    h_off = np.clip(np.floor(ty - tl), 0, H).astype(np.int32)
    w_end = np.clip(np.floor(tx + tl), 0, H).astype(np.int32)
    h_end = np.clip(np.floor(ty + tl), 0, H).astype(np.int32)

    coord = np.arange(H, dtype=np.float32)[None, :]  # [1,448]
    mrow = _sigmoid_f32(10.0 * (coord - w_off[:, None].astype(np.float32))) - _sigmoid_f32(
        10.0 * (coord - w_end[:, None].astype(np.float32))
    )  # [B,448]
    mcol = _sigmoid_f32(10.0 * (coord - h_off[:, None].astype(np.float32))) - _sigmoid_f32(
        10.0 * (coord - h_end[:, None].astype(np.float32))
    )

    t = (np.arange(OUT, dtype=np.float32) / np.float32(OUT - 1))[None, :]  # [1,224]

    def one_hot_lerp(off, end, mask):
        # sample positions r = off + t*(end-1-off), float32 like the reference
        r = off[:, None].astype(np.float32) + t * (end - 1 - off)[:, None].astype(np.float32)
        r0 = np.floor(r).astype(np.int32)
        r1 = np.clip(r0 + 1, 0, H - 1)
        fr = (r - r0).astype(np.float32)
        m = np.zeros((nb, OUT, H), dtype=np.float32)
        bi = np.arange(nb)[:, None]
        oi = np.arange(OUT)[None, :]
        np.add.at(m, (bi, oi, r0), (1.0 - fr).astype(np.float32))
        np.add.at(m, (bi, oi, r1), fr)
        m *= mask[:, None, :]
        return np.ascontiguousarray(m.transpose(0, 2, 1))  # [B,448,224]

    rt = one_hot_lerp(w_off, w_end, mrow)
    cm = one_hot_lerp(h_off, h_end, mcol)
    return rt, cm


# ---------------------------------------------------------------- device code
def _build_nc():
    DT = F32R if USE_F32R else F32
    nc = bacc.Bacc("TRN2", target_bir_lowering=False, debug=False, num_devices=N_CORES)
    img = nc.dram_tensor("images", [BPC, C, H, H], DT, kind="ExternalInput")
    rt = nc.dram_tensor("rt", [BPC, H, OUT], DT, kind="ExternalInput")
    cm = nc.dram_tensor("cm", [BPC, H, OUT], DT, kind="ExternalInput")
    out = nc.dram_tensor("out", [BPC, OUT, C * OUT], F32, kind="ExternalOutput")

    def mm(ap):
        return ap

    with tile.TileContext(nc) as tc, ExitStack() as ctx:
        const_pool = ctx.enter_context(tc.tile_pool(name="const", bufs=1))
        ident_f = const_pool.tile([128, 128], F32)
        make_identity(nc, ident_f)
        if USE_F32R:
            ident = const_pool.tile([128, 128], DT)
            nc.vector.tensor_copy(out=ident, in_=ident_f)
        else:
            ident = ident_f

        img_pool = ctx.enter_context(tc.tile_pool(name="imgp", bufs=2))
        mat_pool = ctx.enter_context(tc.tile_pool(name="matp", bufs=2))
        tmp_pool = ctx.enter_context(tc.tile_pool(name="tmpp", bufs=4))
        tT_pool = ctx.enter_context(tc.tile_pool(name="ttp", bufs=2))
        out_pool = ctx.enter_context(tc.tile_pool(name="outp", bufs=2))
        ps1 = ctx.enter_context(tc.tile_pool(name="ps1", bufs=3, space="PSUM"))
        psT = ctx.enter_context(tc.tile_pool(name="psT", bufs=3, space="PSUM"))
        ps2 = ctx.enter_context(tc.tile_pool(name="ps2", bufs=2, space="PSUM"))

        for b in range(BPC):
            img_s = img_pool.tile([P, C, 4, H], DT, tag="img")
            nc.sync.dma_start(
                out=img_s, in_=img[b].rearrange("c (hc p) w -> p c hc w", p=P)
            )
            rt_s = mat_pool.tile([P, 4, OUT], DT, tag="rt")
            nc.sync.dma_start(out=rt_s, in_=rt[b].rearrange("(hc p) i -> p hc i", p=P))
            cm_s = mat_pool.tile([P, 4, OUT], DT, tag="cm")
            nc.sync.dma_start(out=cm_s, in_=cm[b].rearrange("(wc p) j -> p wc j", p=P))

            tT_all = tT_pool.tile([P, 4, C * OUT], DT, tag="tT")  # [w, wc, (c,i)]
            for c in range(C):
                tmp_sb = []
                for ic in range(2):
                    t_ps = ps1.tile([P, H], F32, tag="s1")
                    for hc in range(4):
                        nc.tensor.matmul(
                            t_ps,
                            mm(rt_s[:, hc, ic * P : (ic + 1) * P]),
                            mm(img_s[:, c, hc, :]),
                            start=(hc == 0),
                            stop=(hc == 3),
                        )
                    sb = tmp_pool.tile([P, H], DT, tag="tmp")
                    nc.vector.tensor_copy(out=sb, in_=t_ps)
                    tmp_sb.append(sb)
                for wc in range(4):
                    tp = psT.tile([P, OUT], DT, tag="sT")
                    for ic in range(2):
                        nc.tensor.transpose(
                            tp[:, ic * P : (ic + 1) * P],
                            tmp_sb[ic][:, wc * P : (wc + 1) * P],
                            ident[:P, :P],
                        )
                    nc.scalar.copy(out=tT_all[:, wc, c * OUT : (c + 1) * OUT], in_=tp)

            out_sb = out_pool.tile([P, 2, C * OUT], F32, tag="osb")  # [j, jc, (c,i)]
            NB = C * OUT // 2  # 336
            for jc in range(2):
                for nb_i in range(2):
                    o_ps = ps2.tile([P, NB], F32, tag="s2")
                    for wc in range(4):
                        nc.tensor.matmul(
                            o_ps,
                            mm(cm_s[:, wc, jc * P : (jc + 1) * P]),
                            mm(tT_all[:, wc, nb_i * NB : (nb_i + 1) * NB]),
                            start=(wc == 0),
                            stop=(wc == 3),
                        )
                    nc.vector.tensor_copy(
                        out=out_sb[:, jc, nb_i * NB : (nb_i + 1) * NB], in_=o_ps
                    )
                nc.sync.dma_start(
                    out=out[b, jc * P : (jc + 1) * P, :], in_=out_sb[:, jc, :]
                )

    nc.compile()
    return nc


def _get_nc():
    if "nc" not in _CACHE:
        _CACHE["nc"] = _build_nc()
    return _CACHE["nc"]


# ---------------------------------------------------------------- entry point
def kernel(images, locs, _trace=False):
    images = np.ascontiguousarray(np.asarray(images, dtype=np.float32))
    locs = np.asarray(locs, dtype=np.float32)
    rt, cm = _interp_matrices(locs)
    if USE_F32R:
        # matmuls run in tf32; pre-round operands so results are deterministic
        images = _round_tf32(images)
        rt = _round_tf32(rt)
        cm = _round_tf32(cm)

    nc = _get_nc()
    in_maps = []
    for i in range(N_CORES):
        s = slice(i * BPC, (i + 1) * BPC)
        in_maps.append(
            {
                "images": images[s],
                "rt": np.ascontiguousarray(rt[s]),
                "cm": np.ascontiguousarray(cm[s]),
            }
        )
    br = run_bass_kernel_spmd(nc, in_maps, core_ids=list(range(N_CORES)), trace=_trace)
    outs = np.concatenate([r["out"] for r in br.results], axis=0)  # [64,224,672]
    # out'[b, j, (c,i)] -> out[b, c, i, j]
    result = np.ascontiguousarray(outs.reshape(B, OUT, C, OUT).transpose(0, 2, 3, 1))
    if _trace:
        return result, br
    return result


if __name__ == "__main__":
    imgs = np.random.randn(B, C, H, H).astype(np.float32)
    lcs = (np.random.rand(B, 3) * H).astype(np.float32)
    out = kernel(imgs, lcs)
    print("out", out.shape, out.dtype)
